# revision 7
# baseline (speedup 1.0000x reference)
"""EDAC layer kernel for Trainium2 (8 NeuronCores, batch-sharded SPMD).

Reference semantics (B=32, C=256, K=64, H=W=56; vulnerable_idx == arange(K)):
  valid(x, c)  = min_vals[c] <= x <= max_vals[c]
  channels >= K:  out = x if valid else 0
  channels <  K:  m = main, d = dup
      both valid  -> min(m, d)      (covers m == d too)
      only d      -> d
      only m      -> m
      neither     -> 0

Kernel strategy (per core, 4 batches):
  rows = (batch, channel) pairs on SBUF partitions, H*W on the free dim.
  Per batch-pair (b, b+1) process five [128, HW] tiles:
    A: batch b   channels  64..191   (simple range-zero path)
    B: batch b   channels 192..255 + batch b+1 channels 64..127
    C: batch b+1 channels 128..255
    V: channels 0..63 of both batches (vulnerable, compared against dup)
    D: dup rows for both batches
  Simple path: two scalar_tensor_tensor ops  ((m>=lo)*m, then (m<=hi)*that).
  Vulnerable:  ScalarE relus r1=relu(lo-m), r2=relu(m-hi) (exact zeroness),
               m1 = m + HUGE*(r1+r2) via two stt ops, r = min(m1, d1),
               res = (r < THR) * r.
"""

import os
import sys

for _p in ("/opt/trn_rl_repo", os.path.expanduser("~/.axon_site/_ro/trn_rl_repo")):
    if os.path.isdir(_p) and _p not in sys.path:
        sys.path.insert(0, _p)

import numpy as np

import concourse.bass as bass
import concourse.bacc as bacc
import concourse.mybir as mybir
from concourse.tile import TileContext
from concourse.bass_utils import run_bass_kernel_spmd

F32 = mybir.dt.float32
BF16 = mybir.dt.bfloat16
OP = mybir.AluOpType
AF = mybir.ActivationFunctionType

B, C, K, H, W = 32, 256, 64, 56, 56
HW = H * W
NCORES = 8
BL = B // NCORES  # batches per core

HUGE = 1.0e30  # sentinel multiplier: HUGE * smallest-positive-bf16-relu >> THR
THR = 1.0e15   # valid values are <= ~10; invalid sentinels are >= ~6e22

# bounds table columns (per-partition scalars for each tile kind)
#   0..3  : lo  for tile kinds A, B, C, V
#   4..7  : hi  for tile kinds A, B, C, V
#   8..11 : -hi for tile kinds A, B, C, V
NBCOLS = 12


def build_bounds(min_vals: np.ndarray, max_vals: np.ndarray) -> np.ndarray:
    lo = np.asarray(min_vals, dtype=np.float32)
    hi = np.asarray(max_vals, dtype=np.float32)
    cols = np.zeros((128, NBCOLS), dtype=np.float32)
    kinds = [
        np.arange(64, 192),                                  # A
        np.concatenate([np.arange(192, 256), np.arange(64, 128)]),  # B
        np.arange(128, 256),                                 # C
        np.concatenate([np.arange(0, 64), np.arange(0, 64)]),       # V
    ]
    for j, idx in enumerate(kinds):
        cols[:, j] = lo[idx]
        cols[:, 4 + j] = hi[idx]
        cols[:, 8 + j] = -hi[idx]
    return cols


def build_nc(hw: int = HW) -> bass.Bass:
    nc = bacc.Bacc("TRN2", target_bir_lowering=False, debug=False)
    R = BL * C
    main = nc.dram_tensor("main", [R, hw], F32, kind="ExternalInput")
    dup = nc.dram_tensor("dup", [BL * K, hw], F32, kind="ExternalInput")
    bounds = nc.dram_tensor("bounds", [128, NBCOLS], F32, kind="ExternalInput")
    out = nc.dram_tensor("out", [R, hw], F32, kind="ExternalOutput")

    stt = nc.vector.scalar_tensor_tensor

    # row-group views for multi-segment (strided) single-start DMAs
    main_g = main.ap().rearrange("(a b) w -> a b w", b=64)   # [BL*4, 64, hw]
    out_g = out.ap().rearrange("(a b) w -> a b w", b=64)

    with TileContext(nc) as tc:
        with (
            tc.tile_pool(name="bnd", bufs=1) as bpool,
            tc.tile_pool(name="pm", bufs=6) as pm,
            tc.tile_pool(name="pv", bufs=2) as pv,
            tc.tile_pool(name="pd", bufs=2) as pd,
            tc.tile_pool(name="pr", bufs=8) as pr,
        ):
            bt = bpool.tile([128, NBCOLS], F32)
            nc.sync.dma_start(out=bt[:], in_=bounds[:])

            def lo_ap(j):
                return bt[:, j:j + 1]

            def hi_ap(j):
                return bt[:, 4 + j:5 + j]

            def nhi_ap(j):
                return bt[:, 8 + j:9 + j]

            for p in range(BL // 2):
                g = p * 8             # first 64-row group of batch b=2p
                dbase = p * 2 * K     # first row of batch b=2p in dup

                # --- vulnerable tile first (longest dependency chain) ---
                # channels 0..63 of batches b and b+1: row groups g, g+4
                mv = pv.tile([128, hw], F32, tag="mv")
                nc.sync.dma_start(out=mv[:], in_=main_g[g:g + 5:4])
                dv = pd.tile([128, hw], F32, tag="dv")
                nc.sync.dma_start(out=dv[:], in_=dup[dbase:dbase + 128, :])

                r1m = pr.tile([128, hw], BF16, tag="rl")
                r2m = pr.tile([128, hw], BF16, tag="rl")
                r1d = pr.tile([128, hw], BF16, tag="rl")
                r2d = pr.tile([128, hw], BF16, tag="rl")
                # r1 = relu(lo - x), r2 = relu(x - hi): >0 iff x invalid (exact)
                nc.scalar.activation(r1m[:], mv[:], AF.Relu, bias=lo_ap(3), scale=-1.0)
                nc.scalar.activation(r2m[:], mv[:], AF.Relu, bias=nhi_ap(3), scale=1.0)
                nc.scalar.activation(r1d[:], dv[:], AF.Relu, bias=lo_ap(3), scale=-1.0)
                nc.scalar.activation(r2d[:], dv[:], AF.Relu, bias=nhi_ap(3), scale=1.0)

                # mv += HUGE*r1m; mv += HUGE*r2m   (sentinel if m invalid)
                stt(out=mv[:], in0=r1m[:], scalar=HUGE, in1=mv[:],
                    op0=OP.mult, op1=OP.add)
                stt(out=mv[:], in0=r2m[:], scalar=HUGE, in1=mv[:],
                    op0=OP.mult, op1=OP.add)
                stt(out=dv[:], in0=r1d[:], scalar=HUGE, in1=dv[:],
                    op0=OP.mult, op1=OP.add)
                stt(out=dv[:], in0=r2d[:], scalar=HUGE, in1=dv[:],
                    op0=OP.mult, op1=OP.add)
                # mv = min(mv, dv); res = (mv < THR) * mv  (into dv)
                nc.vector.tensor_tensor(out=mv[:], in0=mv[:], in1=dv[:], op=OP.min)
                stt(out=dv[:], in0=mv[:], scalar=THR, in1=mv[:],
                    op0=OP.is_lt, op1=OP.mult)
                nc.scalar.dma_start(out=out_g[g:g + 5:4], in_=dv[:])

                # --- simple tiles: (kind, grouped row slice) ---
                simple = [
                    (0, slice(g + 1, g + 3)),       # A: ch 64..191 of b
                    (1, slice(g + 3, g + 6, 2)),    # B: ch 192..255 b, 64..127 b+1
                    (2, slice(g + 6, g + 8)),       # C: ch 128..255 of b+1
                ]
                for kind, sl in simple:
                    mt = pm.tile([128, hw], F32, tag="mt")
                    nc.sync.dma_start(out=mt[:], in_=main_g[sl])
                    # mt = (m >= lo) * m ; then mt = (mt <= hi) * mt
                    # (safe on the masked tile: 0 is always <= hi)
                    stt(out=mt[:], in0=mt[:], scalar=lo_ap(kind), in1=mt[:],
                        op0=OP.is_ge, op1=OP.mult)
                    stt(out=mt[:], in0=mt[:], scalar=hi_ap(kind), in1=mt[:],
                        op0=OP.is_le, op1=OP.mult)
                    nc.scalar.dma_start(out=out_g[sl], in_=mt[:])
    return nc


_NC_CACHE: dict = {}


def _get_nc(hw: int) -> bass.Bass:
    if hw not in _NC_CACHE:
        nc = build_nc(hw)
        nc.finalize()  # Bacc.finalize runs compile() (register allocation etc.)
        _NC_CACHE[hw] = nc
    return _NC_CACHE[hw]


def kernel(main_out, dup_out, min_vals, max_vals, vulnerable_idx):
    return _run(main_out, dup_out, min_vals, max_vals, vulnerable_idx)[0]


def _run(main_out, dup_out, min_vals, max_vals, vulnerable_idx, **spmd_kwargs):
    main_out = np.asarray(main_out)
    dup_out = np.asarray(dup_out)
    min_vals = np.asarray(min_vals)
    max_vals = np.asarray(max_vals)
    vidx = np.asarray(vulnerable_idx).ravel()

    # Device kernel assumes vulnerable channels are 0..K-1. If not, permute
    # channels host-side so they are, and invert on the way out.
    perm = None
    if not np.array_equal(vidx, np.arange(K)):
        assert len(np.unique(vidx)) == K, "duplicate vulnerable_idx unsupported"
        rest = np.setdiff1d(np.arange(C), vidx)
        perm = np.concatenate([vidx, rest])
        main_out = main_out[:, perm]
        min_vals = min_vals[perm]
        max_vals = max_vals[perm]

    mo = np.ascontiguousarray(main_out, dtype=np.float32).reshape(B, C, HW)
    du = np.ascontiguousarray(dup_out, dtype=np.float32).reshape(B, K, HW)
    bounds = build_bounds(min_vals, max_vals)

    in_maps = []
    for k in range(NCORES):
        in_maps.append({
            "main": mo[BL * k:BL * (k + 1)].reshape(BL * C, HW),
            "dup": du[BL * k:BL * (k + 1)].reshape(BL * K, HW),
            "bounds": bounds,
        })

    nc = _get_nc(HW)
    res = run_bass_kernel_spmd(nc, in_maps, list(range(NCORES)), **spmd_kwargs)
    out = np.concatenate(
        [r["out"].reshape(BL, C, H, W) for r in res.results], axis=0)

    if perm is not None:
        inv = np.empty(C, dtype=np.int64)
        inv[perm] = np.arange(C)
        out = out[:, inv]
    return out, res


# revision 9
# speedup vs baseline: 2.5060x; 2.5060x over previous
"""EDAC layer kernel for Trainium2 (8 NeuronCores, batch-sharded SPMD).

Reference semantics (B=32, C=256, K=64, H=W=56; vulnerable_idx == arange(K)):
  valid(x, c)  = min_vals[c] <= x <= max_vals[c]
  channels >= K:  out = x if valid else 0
  channels <  K:  m = main, d = dup
      both valid  -> min(m, d)      (covers m == d too)
      only d      -> d
      only m      -> m
      neither     -> 0

Kernel strategy (per core, 4 batches):
  rows = (batch, channel) pairs on SBUF partitions, H*W on the free dim.
  Per batch-pair (b, b+1) process five [128, HW] tiles:
    A: batch b   channels  64..191   (simple range-zero path)
    B: batch b   channels 192..255 + batch b+1 channels 64..127
    C: batch b+1 channels 128..255
    V: channels 0..63 of both batches (vulnerable, compared against dup)
    D: dup rows for both batches
  Simple path: two scalar_tensor_tensor ops  ((m>=lo)*m, then (m<=hi)*that).
  Vulnerable:  ScalarE relus r1=relu(lo-m), r2=relu(m-hi) (exact zeroness),
               m1 = m + HUGE*(r1+r2) via two stt ops, r = min(m1, d1),
               res = (r < THR) * r.
"""

import os
import sys

for _p in ("/opt/trn_rl_repo", os.path.expanduser("~/.axon_site/_ro/trn_rl_repo")):
    if os.path.isdir(_p) and _p not in sys.path:
        sys.path.insert(0, _p)

import numpy as np

import concourse.bass as bass
import concourse.bacc as bacc
import concourse.mybir as mybir
from concourse.tile import TileContext
from concourse.bass_utils import run_bass_kernel_spmd

F32 = mybir.dt.float32
BF16 = mybir.dt.bfloat16
OP = mybir.AluOpType
AF = mybir.ActivationFunctionType

B, C, K, H, W = 32, 256, 64, 56, 56
HW = H * W
NCORES = 8
BL = B // NCORES  # batches per core

HUGE = 1.0e30  # sentinel multiplier: HUGE * smallest-positive-bf16-relu >> THR
THR = 1.0e15   # valid values are <= ~10; invalid sentinels are >= ~6e22

# bounds table columns (per-partition scalars for each tile kind)
#   0..3  : lo  for tile kinds A, B, C, V
#   4..7  : hi  for tile kinds A, B, C, V
#   8..11 : -hi for tile kinds A, B, C, V
NBCOLS = 12


def build_bounds(min_vals: np.ndarray, max_vals: np.ndarray) -> np.ndarray:
    lo = np.asarray(min_vals, dtype=np.float32)
    hi = np.asarray(max_vals, dtype=np.float32)
    cols = np.zeros((128, NBCOLS), dtype=np.float32)
    kinds = [
        np.arange(64, 192),                                  # A
        np.concatenate([np.arange(192, 256), np.arange(64, 128)]),  # B
        np.arange(128, 256),                                 # C
        np.concatenate([np.arange(0, 64), np.arange(0, 64)]),       # V
    ]
    for j, idx in enumerate(kinds):
        cols[:, j] = lo[idx]
        cols[:, 4 + j] = hi[idx]
        cols[:, 8 + j] = -hi[idx]
    return cols


def build_nc(hw: int = HW) -> bass.Bass:
    nc = bacc.Bacc("TRN2", target_bir_lowering=False, debug=False)
    R = BL * C
    main = nc.dram_tensor("main", [R, hw], F32, kind="ExternalInput")
    dup = nc.dram_tensor("dup", [BL * K, hw], F32, kind="ExternalInput")
    bounds = nc.dram_tensor("bounds", [128, NBCOLS], F32, kind="ExternalInput")
    out = nc.dram_tensor("out", [R, hw], F32, kind="ExternalOutput")

    stt = nc.vector.scalar_tensor_tensor

    # row-group views for multi-segment (strided) single-start DMAs
    main_g = main.ap().rearrange("(a b) w -> a b w", b=64)   # [BL*4, 64, hw]
    out_g = out.ap().rearrange("(a b) w -> a b w", b=64)

    with TileContext(nc) as tc:
        with (
            tc.tile_pool(name="bnd", bufs=1) as bpool,
            tc.tile_pool(name="pm", bufs=6) as pm,
            tc.tile_pool(name="pv", bufs=2) as pv,
            tc.tile_pool(name="pd", bufs=2) as pd,
            tc.tile_pool(name="pr", bufs=8) as pr,
        ):
            bt = bpool.tile([128, NBCOLS], F32)
            nc.sync.dma_start(out=bt[:], in_=bounds[:])

            def lo_ap(j):
                return bt[:, j:j + 1]

            def hi_ap(j):
                return bt[:, 4 + j:5 + j]

            def nhi_ap(j):
                return bt[:, 8 + j:9 + j]

            for p in range(BL // 2):
                g = p * 8             # first 64-row group of batch b=2p
                dbase = p * 2 * K     # first row of batch b=2p in dup

                # --- vulnerable tile first (longest dependency chain) ---
                # channels 0..63 of batches b and b+1: row groups g, g+4
                # (two 2D DMAs: multi-segment 3D APs split across only 2 of
                # the 16 SDMA engines — measured 2.2x slowdown)
                mv = pv.tile([128, hw], F32, tag="mv")
                nc.sync.dma_start(out=mv[0:64, :], in_=main_g[g])
                nc.sync.dma_start(out=mv[64:128, :], in_=main_g[g + 4])
                dv = pd.tile([128, hw], F32, tag="dv")
                nc.sync.dma_start(out=dv[:], in_=dup[dbase:dbase + 128, :])

                r1m = pr.tile([128, hw], BF16, tag="rl")
                r2m = pr.tile([128, hw], BF16, tag="rl")
                r1d = pr.tile([128, hw], BF16, tag="rl")
                r2d = pr.tile([128, hw], BF16, tag="rl")
                # r1 = relu(lo - x), r2 = relu(x - hi): >0 iff x invalid (exact)
                nc.scalar.activation(r1m[:], mv[:], AF.Relu, bias=lo_ap(3), scale=-1.0)
                nc.scalar.activation(r2m[:], mv[:], AF.Relu, bias=nhi_ap(3), scale=1.0)
                nc.scalar.activation(r1d[:], dv[:], AF.Relu, bias=lo_ap(3), scale=-1.0)
                nc.scalar.activation(r2d[:], dv[:], AF.Relu, bias=nhi_ap(3), scale=1.0)

                # mv += HUGE*r1m; mv += HUGE*r2m   (sentinel if m invalid)
                stt(out=mv[:], in0=r1m[:], scalar=HUGE, in1=mv[:],
                    op0=OP.mult, op1=OP.add)
                stt(out=mv[:], in0=r2m[:], scalar=HUGE, in1=mv[:],
                    op0=OP.mult, op1=OP.add)
                stt(out=dv[:], in0=r1d[:], scalar=HUGE, in1=dv[:],
                    op0=OP.mult, op1=OP.add)
                stt(out=dv[:], in0=r2d[:], scalar=HUGE, in1=dv[:],
                    op0=OP.mult, op1=OP.add)
                # mv = min(mv, dv); res = (mv < THR) * mv  (into dv)
                nc.vector.tensor_tensor(out=mv[:], in0=mv[:], in1=dv[:], op=OP.min)
                stt(out=dv[:], in0=mv[:], scalar=THR, in1=mv[:],
                    op0=OP.is_lt, op1=OP.mult)
                nc.scalar.dma_start(out=out_g[g], in_=dv[0:64, :])
                nc.scalar.dma_start(out=out_g[g + 4], in_=dv[64:128, :])

                # --- simple tiles: (kind, [64-row groups]) ---
                simple = [
                    (0, [g + 1, g + 2]),       # A: ch 64..191 of b
                    (1, [g + 3, g + 5]),       # B: ch 192..255 b, 64..127 b+1
                    (2, [g + 6, g + 7]),       # C: ch 128..255 of b+1
                ]
                for kind, groups in simple:
                    mt = pm.tile([128, hw], F32, tag="mt")
                    if groups[1] == groups[0] + 1:  # contiguous pair
                        nc.sync.dma_start(
                            out=mt[:], in_=main_g[groups[0]:groups[0] + 2])
                    else:
                        nc.sync.dma_start(out=mt[0:64, :], in_=main_g[groups[0]])
                        nc.sync.dma_start(out=mt[64:128, :], in_=main_g[groups[1]])
                    # mt = (m >= lo) * m ; then mt = (mt <= hi) * mt
                    # (safe on the masked tile: 0 is always <= hi)
                    stt(out=mt[:], in0=mt[:], scalar=lo_ap(kind), in1=mt[:],
                        op0=OP.is_ge, op1=OP.mult)
                    stt(out=mt[:], in0=mt[:], scalar=hi_ap(kind), in1=mt[:],
                        op0=OP.is_le, op1=OP.mult)
                    if groups[1] == groups[0] + 1:
                        nc.scalar.dma_start(
                            out=out_g[groups[0]:groups[0] + 2], in_=mt[:])
                    else:
                        nc.scalar.dma_start(out=out_g[groups[0]], in_=mt[0:64, :])
                        nc.scalar.dma_start(out=out_g[groups[1]], in_=mt[64:128, :])
    return nc


_NC_CACHE: dict = {}


def _get_nc(hw: int) -> bass.Bass:
    if hw not in _NC_CACHE:
        nc = build_nc(hw)
        nc.finalize()  # Bacc.finalize runs compile() (register allocation etc.)
        _NC_CACHE[hw] = nc
    return _NC_CACHE[hw]


def kernel(main_out, dup_out, min_vals, max_vals, vulnerable_idx):
    return _run(main_out, dup_out, min_vals, max_vals, vulnerable_idx)[0]


def _run(main_out, dup_out, min_vals, max_vals, vulnerable_idx, **spmd_kwargs):
    main_out = np.asarray(main_out)
    dup_out = np.asarray(dup_out)
    min_vals = np.asarray(min_vals)
    max_vals = np.asarray(max_vals)
    vidx = np.asarray(vulnerable_idx).ravel()

    # Device kernel assumes vulnerable channels are 0..K-1. If not, permute
    # channels host-side so they are, and invert on the way out.
    perm = None
    if not np.array_equal(vidx, np.arange(K)):
        assert len(np.unique(vidx)) == K, "duplicate vulnerable_idx unsupported"
        rest = np.setdiff1d(np.arange(C), vidx)
        perm = np.concatenate([vidx, rest])
        main_out = main_out[:, perm]
        min_vals = min_vals[perm]
        max_vals = max_vals[perm]

    mo = np.ascontiguousarray(main_out, dtype=np.float32).reshape(B, C, HW)
    du = np.ascontiguousarray(dup_out, dtype=np.float32).reshape(B, K, HW)
    bounds = build_bounds(min_vals, max_vals)

    in_maps = []
    for k in range(NCORES):
        in_maps.append({
            "main": mo[BL * k:BL * (k + 1)].reshape(BL * C, HW),
            "dup": du[BL * k:BL * (k + 1)].reshape(BL * K, HW),
            "bounds": bounds,
        })

    nc = _get_nc(HW)
    res = run_bass_kernel_spmd(nc, in_maps, list(range(NCORES)), **spmd_kwargs)
    out = np.concatenate(
        [r["out"].reshape(BL, C, H, W) for r in res.results], axis=0)

    if perm is not None:
        inv = np.empty(C, dtype=np.int64)
        inv[perm] = np.arange(C)
        out = out[:, inv]
    return out, res


# revision 10
# speedup vs baseline: 2.7497x; 1.0973x over previous
"""EDAC layer kernel for Trainium2 (8 NeuronCores, batch-sharded SPMD).

Reference semantics (B=32, C=256, K=64, H=W=56; vulnerable_idx == arange(K)):
  valid(x, c)  = min_vals[c] <= x <= max_vals[c]
  channels >= K:  out = x if valid else 0
  channels <  K:  m = main, d = dup
      both valid  -> min(m, d)      (covers m == d too)
      only d      -> d
      only m      -> m
      neither     -> 0

Kernel strategy (per core, 4 batches):
  rows = (batch, channel) pairs on SBUF partitions, H*W on the free dim.
  Per batch-pair (b, b+1) process five [128, HW] tiles:
    A: batch b   channels  64..191   (simple range-zero path)
    B: batch b   channels 192..255 + batch b+1 channels 64..127
    C: batch b+1 channels 128..255
    V: channels 0..63 of both batches (vulnerable, compared against dup)
    D: dup rows for both batches
  Simple path: two scalar_tensor_tensor ops  ((m>=lo)*m, then (m<=hi)*that).
  Vulnerable:  ScalarE relus r1=relu(lo-m), r2=relu(m-hi) (exact zeroness),
               m1 = m + HUGE*(r1+r2) via two stt ops, r = min(m1, d1),
               res = (r < THR) * r.
"""

import os
import sys

for _p in ("/opt/trn_rl_repo", os.path.expanduser("~/.axon_site/_ro/trn_rl_repo")):
    if os.path.isdir(_p) and _p not in sys.path:
        sys.path.insert(0, _p)

import numpy as np

import concourse.bass as bass
import concourse.bacc as bacc
import concourse.mybir as mybir
from concourse.tile import TileContext
from concourse.bass_utils import run_bass_kernel_spmd

F32 = mybir.dt.float32
BF16 = mybir.dt.bfloat16
OP = mybir.AluOpType
AF = mybir.ActivationFunctionType

B, C, K, H, W = 32, 256, 64, 56, 56
HW = H * W
NCORES = 8
BL = B // NCORES  # batches per core

HUGE = 1.0e30  # sentinel multiplier: HUGE * smallest-positive-bf16-relu >> THR
THR = 1.0e15   # valid values are <= ~10; invalid sentinels are >= ~6e22

# bounds table columns (per-partition scalars for each tile kind)
#   0..3  : lo  for tile kinds A, B, C, V
#   4..7  : hi  for tile kinds A, B, C, V
#   8..11 : -hi for tile kinds A, B, C, V
NBCOLS = 12


def build_bounds(min_vals: np.ndarray, max_vals: np.ndarray) -> np.ndarray:
    lo = np.asarray(min_vals, dtype=np.float32)
    hi = np.asarray(max_vals, dtype=np.float32)
    cols = np.zeros((128, NBCOLS), dtype=np.float32)
    interleave = lambda a, b: np.stack([a, b], axis=1).ravel()
    kinds = [
        np.arange(64, 192),                                   # A: ch 64..191
        interleave(np.arange(192, 256), np.arange(64, 128)),  # B (interleaved)
        np.arange(128, 256),                                  # C: ch 128..255
        np.repeat(np.arange(0, 64), 2),                       # V (interleaved)
    ]
    for j, idx in enumerate(kinds):
        cols[:, j] = lo[idx]
        cols[:, 4 + j] = hi[idx]
        cols[:, 8 + j] = -hi[idx]
    return cols


def build_nc(hw: int = HW) -> bass.Bass:
    nc = bacc.Bacc("TRN2", target_bir_lowering=False, debug=False)
    R = BL * C
    main = nc.dram_tensor("main", [R, hw], F32, kind="ExternalInput")
    dup = nc.dram_tensor("dup", [BL * K, hw], F32, kind="ExternalInput")
    bounds = nc.dram_tensor("bounds", [128, NBCOLS], F32, kind="ExternalInput")
    out = nc.dram_tensor("out", [R, hw], F32, kind="ExternalOutput")

    stt = nc.vector.scalar_tensor_tensor
    npairs = BL // 2

    # Per-pair DRAM views. B and V tiles interleave their two 64-row segments
    # into even/odd SBUF partitions via a [64, 2, hw] AP (outer dim 64), so a
    # single dma_start still spreads over all 16 SDMA engines with full
    # 128-partition port coverage (64-partition DMAs run at half BW; multi-
    # segment outer-dim-2 APs collapse onto 2 engines).
    main_p = main.ap().rearrange("(p x) w -> p x w", p=npairs)   # [p, 512, hw]
    out_p = out.ap().rearrange("(p x) w -> p x w", p=npairs)
    dup_p = dup.ap().rearrange("(p s c) w -> p c s w", p=npairs, s=2)

    def v_ap(t):   # [64, 2, hw]: ch 0..63 of batches b, b+1 interleaved
        return t.rearrange("(s g c) w -> g c s w", s=2, g=4)[0]

    def b_ap(t):   # [64, 2, hw]: ch 192..255 of b / ch 64..127 of b+1
        return t[192:384].rearrange("(s c) w -> c s w", s=3)[:, 0:3:2]

    APS = {
        0: lambda t: t[64:192],      # A
        1: b_ap,                     # B
        2: lambda t: t[384:512],     # C
    }

    with TileContext(nc) as tc:
        with (
            tc.tile_pool(name="bnd", bufs=1) as bpool,
            tc.tile_pool(name="pm", bufs=6) as pm,
            tc.tile_pool(name="pv", bufs=2) as pv,
            tc.tile_pool(name="pd", bufs=2) as pd,
            tc.tile_pool(name="pr", bufs=8) as pr,
        ):
            bt = bpool.tile([128, NBCOLS], F32)
            nc.scalar.dma_start(out=bt[:], in_=bounds[:])

            def lo_ap(j):
                return bt[:, j:j + 1]

            def hi_ap(j):
                return bt[:, 4 + j:5 + j]

            def nhi_ap(j):
                return bt[:, 8 + j:9 + j]

            # all load triggers up-front (scalar HWDGE ring), in the order
            # the consumers need the data
            tiles = []
            for p in range(npairs):
                ma = pm.tile([128, hw], F32, tag="mt")
                nc.scalar.dma_start(out=ma[:], in_=APS[0](main_p[p]))
                mb = pm.tile([128, hw], F32, tag="mt")
                nc.scalar.dma_start(out=mb[:], in_=APS[1](main_p[p]))
                mv = pv.tile([128, hw], F32, tag="mv")
                nc.scalar.dma_start(out=mv[:], in_=v_ap(main_p[p]))
                mc = pm.tile([128, hw], F32, tag="mt")
                nc.scalar.dma_start(out=mc[:], in_=APS[2](main_p[p]))
                dv = pd.tile([128, hw], F32, tag="dv")
                nc.scalar.dma_start(out=dv[:], in_=dup_p[p])
                tiles.append((ma, mb, mc, mv, dv))

            for p in range(npairs):
                ma, mb, mc, mv, dv = tiles[p]
                # ScalarE: r1 = relu(lo - x), r2 = relu(x - hi); invalid iff >0
                r1m = pr.tile([128, hw], BF16, tag="rl")
                r2m = pr.tile([128, hw], BF16, tag="rl")
                r1d = pr.tile([128, hw], BF16, tag="rl")
                r2d = pr.tile([128, hw], BF16, tag="rl")
                nc.scalar.activation(r1m[:], mv[:], AF.Relu, bias=lo_ap(3), scale=-1.0)
                nc.scalar.activation(r2m[:], mv[:], AF.Relu, bias=nhi_ap(3), scale=1.0)
                nc.scalar.activation(r1d[:], dv[:], AF.Relu, bias=lo_ap(3), scale=-1.0)
                nc.scalar.activation(r2d[:], dv[:], AF.Relu, bias=nhi_ap(3), scale=1.0)

                # simple tiles, fully in-place on the input tile
                for kind, mt in ((0, ma), (1, mb), (2, mc)):
                    stt(out=mt[:], in0=mt[:], scalar=lo_ap(kind), in1=mt[:],
                        op0=OP.is_ge, op1=OP.mult)
                    stt(out=mt[:], in0=mt[:], scalar=hi_ap(kind), in1=mt[:],
                        op0=OP.is_le, op1=OP.mult)
                    nc.sync.dma_start(out=APS[kind](out_p[p]), in_=mt[:])

                # vulnerable: w = r1 + r2 (bf16, 2x mode), sentinel, min, zero
                nc.vector.tensor_tensor(out=r1m[:], in0=r1m[:], in1=r2m[:], op=OP.add)
                nc.vector.tensor_tensor(out=r1d[:], in0=r1d[:], in1=r2d[:], op=OP.add)
                stt(out=mv[:], in0=r1m[:], scalar=HUGE, in1=mv[:],
                    op0=OP.mult, op1=OP.add)
                stt(out=dv[:], in0=r1d[:], scalar=HUGE, in1=dv[:],
                    op0=OP.mult, op1=OP.add)
                nc.vector.tensor_tensor(out=mv[:], in0=mv[:], in1=dv[:], op=OP.min)
                stt(out=dv[:], in0=mv[:], scalar=THR, in1=mv[:],
                    op0=OP.is_lt, op1=OP.mult)
                nc.sync.dma_start(out=v_ap(out_p[p]), in_=dv[:])
    return nc


_NC_CACHE: dict = {}


def _get_nc(hw: int) -> bass.Bass:
    if hw not in _NC_CACHE:
        nc = build_nc(hw)
        nc.finalize()  # Bacc.finalize runs compile() (register allocation etc.)
        _NC_CACHE[hw] = nc
    return _NC_CACHE[hw]


def kernel(main_out, dup_out, min_vals, max_vals, vulnerable_idx):
    return _run(main_out, dup_out, min_vals, max_vals, vulnerable_idx)[0]


def _run(main_out, dup_out, min_vals, max_vals, vulnerable_idx, **spmd_kwargs):
    main_out = np.asarray(main_out)
    dup_out = np.asarray(dup_out)
    min_vals = np.asarray(min_vals)
    max_vals = np.asarray(max_vals)
    vidx = np.asarray(vulnerable_idx).ravel()

    # Device kernel assumes vulnerable channels are 0..K-1. If not, permute
    # channels host-side so they are, and invert on the way out.
    perm = None
    if not np.array_equal(vidx, np.arange(K)):
        assert len(np.unique(vidx)) == K, "duplicate vulnerable_idx unsupported"
        rest = np.setdiff1d(np.arange(C), vidx)
        perm = np.concatenate([vidx, rest])
        main_out = main_out[:, perm]
        min_vals = min_vals[perm]
        max_vals = max_vals[perm]

    mo = np.ascontiguousarray(main_out, dtype=np.float32).reshape(B, C, HW)
    du = np.ascontiguousarray(dup_out, dtype=np.float32).reshape(B, K, HW)
    bounds = build_bounds(min_vals, max_vals)

    in_maps = []
    for k in range(NCORES):
        in_maps.append({
            "main": mo[BL * k:BL * (k + 1)].reshape(BL * C, HW),
            "dup": du[BL * k:BL * (k + 1)].reshape(BL * K, HW),
            "bounds": bounds,
        })

    nc = _get_nc(HW)
    res = run_bass_kernel_spmd(nc, in_maps, list(range(NCORES)), **spmd_kwargs)
    out = np.concatenate(
        [r["out"].reshape(BL, C, H, W) for r in res.results], axis=0)

    if perm is not None:
        inv = np.empty(C, dtype=np.int64)
        inv[perm] = np.arange(C)
        out = out[:, inv]
    return out, res


# revision 11
# speedup vs baseline: 2.9053x; 1.0566x over previous
"""EDAC layer kernel for Trainium2 (8 NeuronCores, batch-sharded SPMD).

Reference semantics (B=32, C=256, K=64, H=W=56; vulnerable_idx == arange(K)):
  valid(x, c)  = min_vals[c] <= x <= max_vals[c]
  channels >= K:  out = x if valid else 0
  channels <  K:  m = main, d = dup
      both valid  -> min(m, d)      (covers m == d too)
      only d      -> d
      only m      -> m
      neither     -> 0

Kernel strategy (per core, 4 batches):
  rows = (batch, channel) pairs on SBUF partitions, H*W on the free dim.
  Per batch-pair (b, b+1) process five [128, HW] tiles:
    A: batch b   channels  64..191   (simple range-zero path)
    B: batch b   channels 192..255 + batch b+1 channels 64..127
    C: batch b+1 channels 128..255
    V: channels 0..63 of both batches (vulnerable, compared against dup)
    D: dup rows for both batches
  Simple path: two scalar_tensor_tensor ops  ((m>=lo)*m, then (m<=hi)*that).
  Vulnerable:  ScalarE relus r1=relu(lo-m), r2=relu(m-hi) (exact zeroness),
               m1 = m + HUGE*(r1+r2) via two stt ops, r = min(m1, d1),
               res = (r < THR) * r.
"""

import os
import sys

for _p in ("/opt/trn_rl_repo", os.path.expanduser("~/.axon_site/_ro/trn_rl_repo")):
    if os.path.isdir(_p) and _p not in sys.path:
        sys.path.insert(0, _p)

import numpy as np

import concourse.bass as bass
import concourse.bacc as bacc
import concourse.mybir as mybir
from concourse.tile import TileContext
from concourse.bass_utils import run_bass_kernel_spmd

F32 = mybir.dt.float32
BF16 = mybir.dt.bfloat16
OP = mybir.AluOpType
AF = mybir.ActivationFunctionType

B, C, K, H, W = 32, 256, 64, 56, 56
HW = H * W
NCORES = 8
BL = B // NCORES  # batches per core

HUGE = 1.0e30  # sentinel multiplier: HUGE * smallest-positive-bf16-relu >> THR
THR = 1.0e15   # valid values are <= ~10; invalid sentinels are >= ~6e22

# bounds table columns (per-partition scalars for each tile kind)
#   0..3  : lo  for tile kinds A, B, C, V
#   4..7  : hi  for tile kinds A, B, C, V
#   8..11 : -hi for tile kinds A, B, C, V
NBCOLS = 12


def build_bounds(min_vals: np.ndarray, max_vals: np.ndarray) -> np.ndarray:
    lo = np.asarray(min_vals, dtype=np.float32)
    hi = np.asarray(max_vals, dtype=np.float32)
    cols = np.zeros((128, NBCOLS), dtype=np.float32)
    interleave = lambda a, b: np.stack([a, b], axis=1).ravel()
    kinds = [
        np.arange(64, 192),                                   # A: ch 64..191
        interleave(np.arange(192, 256), np.arange(64, 128)),  # B (interleaved)
        np.arange(128, 256),                                  # C: ch 128..255
        np.repeat(np.arange(0, 64), 2),                       # V (interleaved)
    ]
    for j, idx in enumerate(kinds):
        cols[:, j] = lo[idx]
        cols[:, 4 + j] = hi[idx]
        cols[:, 8 + j] = -hi[idx]
    return cols


def build_nc(hw: int = HW) -> bass.Bass:
    nc = bacc.Bacc("TRN2", target_bir_lowering=False, debug=False)
    R = BL * C
    main = nc.dram_tensor("main", [R, hw], F32, kind="ExternalInput")
    dup = nc.dram_tensor("dup", [BL * K, hw], F32, kind="ExternalInput")
    bounds = nc.dram_tensor("bounds", [128, NBCOLS], F32, kind="ExternalInput")
    out = nc.dram_tensor("out", [R, hw], F32, kind="ExternalOutput")

    stt = nc.vector.scalar_tensor_tensor
    npairs = BL // 2

    # Per-pair DRAM views. B and V tiles interleave their two 64-row segments
    # into even/odd SBUF partitions via a [64, 2, hw] AP (outer dim 64), so a
    # single dma_start still spreads over all 16 SDMA engines with full
    # 128-partition port coverage (64-partition DMAs run at half BW; multi-
    # segment outer-dim-2 APs collapse onto 2 engines).
    main_p = main.ap().rearrange("(p x) w -> p x w", p=npairs)   # [p, 512, hw]
    out_p = out.ap().rearrange("(p x) w -> p x w", p=npairs)
    dup_p = dup.ap().rearrange("(p s c) w -> p c s w", p=npairs, s=2)

    def v_ap(t):   # [64, 2, hw]: ch 0..63 of batches b, b+1 interleaved
        return t.rearrange("(s g c) w -> g c s w", s=2, g=4)[0]

    def b_ap(t):   # [64, 2, hw]: ch 192..255 of b / ch 64..127 of b+1
        return t[192:384].rearrange("(s c) w -> c s w", s=3)[:, 0:3:2]

    APS = {
        0: lambda t: t[64:192],      # A
        1: b_ap,                     # B
        2: lambda t: t[384:512],     # C
    }

    with TileContext(nc) as tc:
        with (
            tc.tile_pool(name="bnd", bufs=1) as bpool,
            tc.tile_pool(name="pm", bufs=6) as pm,
            tc.tile_pool(name="pv", bufs=2) as pv,
            tc.tile_pool(name="pd", bufs=2) as pd,
            tc.tile_pool(name="pr", bufs=8) as pr,
        ):
            bt = bpool.tile([128, NBCOLS], F32)
            nc.scalar.dma_start(out=bt[:], in_=bounds[:])

            def lo_ap(j):
                return bt[:, j:j + 1]

            def hi_ap(j):
                return bt[:, 4 + j:5 + j]

            def nhi_ap(j):
                return bt[:, 8 + j:9 + j]

            # All load triggers up-front (scalar HWDGE ring). V/D tiles of
            # BOTH pairs go first: the vulnerable chain (load -> 4 relus ->
            # w -> sentinel -> min -> zero) is ~27us long, so its inputs must
            # land early; simple tiles have a short chain and can arrive late.
            vd = []
            for p in range(npairs):
                mv = pv.tile([128, hw], F32, tag="mv")
                nc.scalar.dma_start(out=mv[:], in_=v_ap(main_p[p]))
                dv = pd.tile([128, hw], F32, tag="dv")
                nc.scalar.dma_start(out=dv[:], in_=dup_p[p])
                vd.append((mv, dv))
            abc = []
            for p in range(npairs):
                row = []
                for kind in (0, 1, 2):
                    mt = pm.tile([128, hw], F32, tag="mt")
                    nc.scalar.dma_start(out=mt[:], in_=APS[kind](main_p[p]))
                    row.append(mt)
                abc.append(row)

            # ScalarE relu stream: both pairs back-to-back so the last relus
            # finish early.  r1 = relu(lo - x), r2 = relu(x - hi).
            relus = []
            for p in range(npairs):
                mv, dv = vd[p]
                r1m = pr.tile([128, hw], BF16, tag="rl")
                r2m = pr.tile([128, hw], BF16, tag="rl")
                r1d = pr.tile([128, hw], BF16, tag="rl")
                r2d = pr.tile([128, hw], BF16, tag="rl")
                nc.scalar.activation(r1m[:], mv[:], AF.Relu, bias=lo_ap(3), scale=-1.0)
                nc.scalar.activation(r2m[:], mv[:], AF.Relu, bias=nhi_ap(3), scale=1.0)
                nc.scalar.activation(r1d[:], dv[:], AF.Relu, bias=lo_ap(3), scale=-1.0)
                nc.scalar.activation(r2d[:], dv[:], AF.Relu, bias=nhi_ap(3), scale=1.0)
                relus.append((r1m, r2m, r1d, r2d))

            def do_simple(p, kind):
                mt = abc[p][kind]
                stt(out=mt[:], in0=mt[:], scalar=lo_ap(kind), in1=mt[:],
                    op0=OP.is_ge, op1=OP.mult)
                stt(out=mt[:], in0=mt[:], scalar=hi_ap(kind), in1=mt[:],
                    op0=OP.is_le, op1=OP.mult)
                nc.sync.dma_start(out=APS[kind](out_p[p]), in_=mt[:])

            def do_vuln(p):
                mv, dv = vd[p]
                r1m, r2m, r1d, r2d = relus[p]
                # w = r1 + r2 (bf16 2x mode); sentinel; min; zero-invalid
                nc.vector.tensor_tensor(out=r1m[:], in0=r1m[:], in1=r2m[:], op=OP.add)
                nc.vector.tensor_tensor(out=r1d[:], in0=r1d[:], in1=r2d[:], op=OP.add)
                stt(out=mv[:], in0=r1m[:], scalar=HUGE, in1=mv[:],
                    op0=OP.mult, op1=OP.add)
                stt(out=dv[:], in0=r1d[:], scalar=HUGE, in1=dv[:],
                    op0=OP.mult, op1=OP.add)
                nc.vector.tensor_tensor(out=mv[:], in0=mv[:], in1=dv[:], op=OP.min)
                stt(out=dv[:], in0=mv[:], scalar=THR, in1=mv[:],
                    op0=OP.is_lt, op1=OP.mult)
                nc.sync.dma_start(out=v_ap(out_p[p]), in_=dv[:])

            # DVE stream: long vulnerable chains early, simple tiles fill
            do_simple(0, 0)
            do_vuln(0)
            do_simple(0, 1)
            do_simple(0, 2)
            do_simple(1, 0)
            do_vuln(1)
            do_simple(1, 1)
            do_simple(1, 2)
    return nc


_NC_CACHE: dict = {}


def _get_nc(hw: int) -> bass.Bass:
    if hw not in _NC_CACHE:
        nc = build_nc(hw)
        nc.finalize()  # Bacc.finalize runs compile() (register allocation etc.)
        _NC_CACHE[hw] = nc
    return _NC_CACHE[hw]


def kernel(main_out, dup_out, min_vals, max_vals, vulnerable_idx):
    return _run(main_out, dup_out, min_vals, max_vals, vulnerable_idx)[0]


def _run(main_out, dup_out, min_vals, max_vals, vulnerable_idx, **spmd_kwargs):
    main_out = np.asarray(main_out)
    dup_out = np.asarray(dup_out)
    min_vals = np.asarray(min_vals)
    max_vals = np.asarray(max_vals)
    vidx = np.asarray(vulnerable_idx).ravel()

    # Device kernel assumes vulnerable channels are 0..K-1. If not, permute
    # channels host-side so they are, and invert on the way out.
    perm = None
    if not np.array_equal(vidx, np.arange(K)):
        assert len(np.unique(vidx)) == K, "duplicate vulnerable_idx unsupported"
        rest = np.setdiff1d(np.arange(C), vidx)
        perm = np.concatenate([vidx, rest])
        main_out = main_out[:, perm]
        min_vals = min_vals[perm]
        max_vals = max_vals[perm]

    mo = np.ascontiguousarray(main_out, dtype=np.float32).reshape(B, C, HW)
    du = np.ascontiguousarray(dup_out, dtype=np.float32).reshape(B, K, HW)
    bounds = build_bounds(min_vals, max_vals)

    in_maps = []
    for k in range(NCORES):
        in_maps.append({
            "main": mo[BL * k:BL * (k + 1)].reshape(BL * C, HW),
            "dup": du[BL * k:BL * (k + 1)].reshape(BL * K, HW),
            "bounds": bounds,
        })

    nc = _get_nc(HW)
    res = run_bass_kernel_spmd(nc, in_maps, list(range(NCORES)), **spmd_kwargs)
    out = np.concatenate(
        [r["out"].reshape(BL, C, H, W) for r in res.results], axis=0)

    if perm is not None:
        inv = np.empty(C, dtype=np.int64)
        inv[perm] = np.arange(C)
        out = out[:, inv]
    return out, res
